# revision 21
# baseline (speedup 1.0000x reference)
"""TRN2 kernel v5a: chunked causal linear attention, fp8-DR state/cross.

Beyond v2: the fp8 state path measured 1.1e-2 (< 2e-2 gate), so
 - state accumulates TWO chunks per fp8 DoubleRow matmul (kt = sub-chunk,
   K=256); V is still computed by fp16 matmuls but the psum is cast
   straight to fp8 V8 tiles (no fp16 V in SBUF at all) -- computing V
   from fp8 INPUTS instead measured 2.7e-2: quantize outputs, not inputs.
 - odd chunks replace the missing state tap with an unmasked cross
   Omega/intra block pair (exact math), so taps halve and chunk 1 loses
   its inter entirely.
PE: 340 matmuls (240 fp16 + 100 DoubleRow), ~133k columns
(v2: 356 / ~143k).  A PE-stream-only hardware probe ties the full kernel
within 1%: every copy/mask/tap/DMA is hidden and the remaining time is
columns x 0.42ns + ~58ns per matmul of dispatch + weight-load tax
(bass emits InstLdweights 1:1 with matmuls, no dedup).
"""

import numpy as np

N_T = 2048
N_IN = 256
C = 128
NCH = N_T // C      # 16 chunks, 8 pairs
T_TILE = 512
TT = N_T // T_TILE
HL = 4
N_CORES = 8

_cache = {}


def _tri_mask():
    idx = np.arange(128)
    return (idx[None, :] >= idx[:, None]).astype(np.float32)


def _build_nc(repeat=1, bf16="fp16"):
    import concourse.tile as tile
    import concourse.mybir as mybir
    from concourse import bacc

    F32 = mybir.dt.float32
    F8 = mybir.dt.float8e4
    FMM = mybir.dt.float16
    DR = mybir.MatmulPerfMode.DoubleRow

    nc = bacc.Bacc("TRN2", target_bir_lowering=False, debug=False,
                   num_devices=N_CORES)
    rT_d = nc.dram_tensor("rT", (2, 128, N_T), FMM, kind="ExternalInput").ap()
    rT8_d = nc.dram_tensor("rT8", (128, 2, N_T), F8,
                           kind="ExternalInput").ap()
    rn8_d = nc.dram_tensor("rn8", (128, NCH // 2, 2, N_IN), F8,
                           kind="ExternalInput").ap()
    Q4_d = nc.dram_tensor("Q4", (HL, 2, 128, N_IN), FMM,
                          kind="ExternalInput").ap()
    ET4_d = nc.dram_tensor("ET4", (HL, 2, 128, N_IN), FMM,
                           kind="ExternalInput").ap()
    mask_d = nc.dram_tensor("mask", (128, 128), FMM,
                            kind="ExternalInput").ap()
    out_d = nc.dram_tensor("out", (NCH, 128, N_IN), F32,
                           kind="ExternalOutput").ap()

    eng_load = {"v": 0.0, "s": 0.0}

    def copy_psum(out_ap, in_ap, n):
        dve = n / 0.96 + 150.0
        act = (n + 352.0) / 1.2
        if eng_load["v"] + dve <= eng_load["s"] + act:
            eng_load["v"] += dve
            nc.vector.tensor_copy(out_ap, in_ap)
        else:
            eng_load["s"] += act
            nc.scalar.copy(out_ap, in_ap)

    with tile.TileContext(nc) as tc:
        with (
            tc.tile_pool(name="const", bufs=1) as const,
            tc.tile_pool(name="om_pool", bufs=8) as om_pool,
            tc.tile_pool(name="opool", bufs=4) as opool,
            tc.tile_pool(name="psS", bufs=1, space="PSUM") as psS,
            tc.tile_pool(name="pswork", bufs=3, space="PSUM") as pswork,
            tc.tile_pool(name="psout", bufs=1, space="PSUM") as psout,
        ):
            warm_f32 = const.tile([128, 128], F32)
            nc.vector.memset(warm_f32, 0.0)
            warm_sb = const.tile([128, 128], FMM)
            nc.vector.tensor_copy(warm_sb, warm_f32)
            warm_ps = pswork.tile([128, T_TILE], F32, tag="w", name="warm_ps")
            for _w in range(24):
                nc.tensor.matmul(warm_ps[:, :128], warm_sb, warm_sb,
                                 start=True, stop=True, skip_group_check=True)

            mask_sb = const.tile([128, 2, 128], FMM)
            Q_h = [const.tile([128, 2, N_IN], FMM, name=f"Qh{h}")
                   for h in range(HL)]
            rT_sb = [const.tile([128, N_T], FMM, name=f"rT{jc}")
                     for jc in range(2)]
            rT8_sb = const.tile([128, 2, N_T], F8, name="rT8_sb")
            rn8_sb = const.tile([128, NCH // 2, 2, N_IN], F8, name="rn8_sb")
            ET_p = [const.tile([128, 2, 2, N_IN], FMM, name=f"ETp{p}")
                    for p in range(2)]
            for h in range(HL):
                for ic in range(2):
                    nc.sync.dma_start(out=Q_h[h][:, ic, :], in_=Q4_d[h, ic])
            for tq in range(TT):
                for ic in range(2):
                    nc.sync.dma_start(
                        out=rT_sb[ic][:, T_TILE * tq:T_TILE * (tq + 1)],
                        in_=rT_d[ic, :, T_TILE * tq:T_TILE * (tq + 1)])
            for jc in range(2):
                nc.sync.dma_start(out=rT8_sb[:, jc, :], in_=rT8_d[:, jc, :])
            for p in range(2):
                for jc in range(2):
                    for h2 in range(2):
                        nc.sync.dma_start(out=ET_p[p][:, jc, h2, :],
                                          in_=ET4_d[2 * p + h2, jc])
            for h2 in range(2):
                nc.sync.dma_start(out=mask_sb[:, h2, :], in_=mask_d)
            for m in range(NCH // 2):
                nc.sync.dma_start(out=rn8_sb[:, m, :, :], in_=rn8_d[:, m])

            # P^T fp16 [j, h2, n, t] (inter) + fp8 shadow [j, jc, h, n, t]
            PTp = [[const.tile([128, 2, NCH, C], FMM, name=f"PTp{p}_{jc}")
                    for jc in range(2)] for p in range(2)]
            PT8 = const.tile([128, 2, HL, NCH, C], F8, name="PT8")
            # V8 per (pair-of-chunks m, head-pair p): [u, kt=n%2, h2, i]
            V8_t = [[const.tile([128, 2, 2, N_IN], F8, name=f"V8{m}_{p}")
                     for p in range(2)] for m in range(NCH // 2)]
            S_ps = [[psS.tile([128, 2, N_IN], F32, name=f"Sps{p}_{jb}")
                     for jb in range(2)] for p in range(2)]
            S_sb = [[[const.tile([128, 2, N_IN], FMM, name=f"Ssb{p}_{jb}_{b}")
                      for b in range(2)] for jb in range(2)] for p in range(2)]

            def body():
                # ---- Phase A: P^T = Q^T rT (fp16, 2-pass over ic)
                for tq in range(TT):
                    ts = slice(T_TILE * tq, T_TILE * (tq + 1))
                    for p in range(2):
                        for h2 in range(2):
                            h = 2 * p + h2
                            for jc in range(2):
                                ps = pswork.tile([128, T_TILE], F32, tag="w",
                                                 name="ps_a")
                                for ic in range(2):
                                    nc.tensor.matmul(
                                        ps,
                                        Q_h[h][:, ic, 128 * jc:128 * (jc + 1)],
                                        rT_sb[ic][:, ts],
                                        start=(ic == 0), stop=(ic == 1))
                                copy_psum(
                                    PTp[p][jc][:, h2, 4 * tq:4 * tq + 4, :],
                                    ps, T_TILE)
                                copy_psum(
                                    PT8[:, jc, 2 * p + h2,
                                        4 * tq:4 * tq + 4, :],
                                    ps, T_TILE)
                # ---- Phase V: fp16 matmuls, psum cast straight to V8
                for n in range(NCH):
                    cs = slice(C * n, C * (n + 1))
                    pv = [pswork.tile([128, 2, N_IN], F32, tag="w",
                                      name=f"ps_v{p}") for p in range(2)]
                    for jc in range(2):
                        for p in range(2):
                            nc.tensor.matmul(pv[p], rT_sb[jc][:, cs],
                                             ET_p[p][:, jc, :, :],
                                             start=(jc == 0), stop=(jc == 1),
                                             skip_group_check=True)
                    for p in range(2):
                        copy_psum(V8_t[n // 2][p][:, n % 2, :, :], pv[p],
                                  2 * N_IN)
                # ---- Main loop ----
                for n in range(NCH):
                    cs = slice(C * n, C * (n + 1))
                    m = n // 2
                    odd = n % 2 == 1
                    # diag OmT (all 4 heads, one DR matmul)
                    w = pswork.tile([128, HL, C], F32, tag="w", name="ps_om")
                    nc.tensor.matmul(w, rT8_sb[:, :, cs],
                                     PT8[:, :, :, n, :],
                                     start=True, stop=True, perf_mode=DR,
                                     skip_group_check=True)
                    om = []
                    for p in range(2):
                        o = om_pool.tile([128, 2, C], F8, tag="om", name="om")
                        nc.vector.tensor_mul(o, w[:, 2 * p:2 * p + 2, :],
                                             mask_sb)
                        eng_load["v"] += 256 / 0.96 + 150.0
                        om.append(o)
                    if odd:  # cross OmT vs previous chunk's u (unmasked)
                        ps = slice(C * (n - 1), C * n)
                        wc = pswork.tile([128, HL, C], F32, tag="w",
                                         name="ps_omc")
                        nc.tensor.matmul(wc, rT8_sb[:, :, ps],
                                         PT8[:, :, :, n, :],
                                         start=True, stop=True, perf_mode=DR,
                                         skip_group_check=True)
                        omc = []
                        for p in range(2):
                            o = om_pool.tile([128, 2, C], F8, tag="om",
                                             name="omc")
                            copy_psum(o, wc[:, 2 * p:2 * p + 2, :], 2 * C)
                            omc.append(o)
                    po = psout.tile([128, N_IN], F32, tag="po", name="po")
                    n_mm = 2 + (8 if n >= 2 else 0) + (2 if odd else 0)
                    k = 0
                    if n >= 2:  # inter vs tap of pair (n-2)//2
                        bu = ((n - 2) // 2) % 2
                        for p in range(2):
                            for h2 in range(2):
                                for jc in range(2):
                                    nc.tensor.matmul(
                                        po, PTp[p][jc][:, h2, n, :],
                                        S_sb[p][jc][bu][:, h2, :],
                                        start=(k == 0), stop=(k == n_mm - 1),
                                        skip_group_check=True)
                                    k += 1
                    for p in range(2):  # diag intra (DR head-pair sum)
                        nc.tensor.matmul(po, om[p], V8_t[m][p][:, n % 2, :, :],
                                         start=(k == 0), stop=(k == n_mm - 1),
                                         perf_mode=DR, skip_group_check=True)
                        k += 1
                    if odd:  # cross intra vs previous chunk's V
                        for p in range(2):
                            nc.tensor.matmul(po, omc[p],
                                             V8_t[m][p][:, 0, :, :],
                                             start=(k == 0),
                                             stop=(k == n_mm - 1),
                                             perf_mode=DR,
                                             skip_group_check=True)
                            k += 1
                    # state: add pair (n-1, n) at odd chunks 1..13 (DR, K=256)
                    if odd and n < NCH - 1:
                        for jb in range(2):
                            for p in range(2):
                                nc.tensor.matmul(
                                    S_ps[p][jb],
                                    rn8_sb[:, m, :, 128 * jb:128 * (jb + 1)],
                                    V8_t[m][p][:, :, :, :],
                                    start=(m == 0), stop=True,
                                    perf_mode=DR, skip_group_check=True)
                        for p in range(2):
                            for jb in range(2):
                                copy_psum(S_sb[p][jb][m % 2], S_ps[p][jb],
                                          2 * N_IN)
                    ot = opool.tile([128, N_IN], F32, tag="ot", name="ot")
                    copy_psum(ot, po, N_IN)
                    nc.sync.dma_start(out=out_d[n], in_=ot)

            if repeat == 1:
                body()
            elif isinstance(repeat, tuple):
                loop_n, unroll_m = repeat
                with tc.For_i(0, loop_n, 1):
                    for _ in range(unroll_m):
                        body()
            elif repeat < 0:
                for _ in range(-repeat):
                    body()
            else:
                with tc.For_i(0, repeat, 1):
                    body()
    nc.compile()
    return nc


DTYPE = "fp16"


def _prep_in_maps(r_prime, E, Q, bf16="fp16"):
    import ml_dtypes
    f8 = ml_dtypes.float8_e4m3
    f16 = np.float16
    mask = _tri_mask()
    in_maps = []
    for c in range(N_CORES):
        b, hg = divmod(c, 2)
        heads = slice(4 * hg, 4 * hg + 4)
        rT = np.ascontiguousarray(r_prime[0, b].T).reshape(2, 128, N_T)
        rT8 = np.ascontiguousarray(rT.transpose(1, 0, 2))
        rn8 = np.ascontiguousarray(
            r_prime[0, b].reshape(NCH // 2, 2, 128, N_IN)
            .transpose(2, 0, 1, 3))
        Q4 = np.ascontiguousarray(Q[0, heads]).reshape(HL, 2, 128, N_IN)
        ET4 = np.ascontiguousarray(
            E[0, heads].transpose(0, 2, 1)).reshape(HL, 2, 128, N_IN)
        in_maps.append({"rT": rT.astype(f16),
                        "rT8": rT8.astype(f8),
                        "rn8": rn8.astype(f8),
                        "Q4": Q4.astype(f16),
                        "ET4": ET4.astype(f16),
                        "mask": mask.astype(f16)})
    return in_maps


def kernel(r_prime, E, Q):
    from concourse import bass_utils

    if "nc" not in _cache:
        _cache["nc"] = _build_nc(bf16=DTYPE)
    nc = _cache["nc"]
    in_maps = _prep_in_maps(r_prime, E, Q, bf16=DTYPE)
    res = bass_utils.run_bass_kernel_spmd(nc, in_maps,
                                          core_ids=list(range(N_CORES)))
    out = np.zeros((1, 4, N_T, N_IN), dtype=np.float32)
    for b in range(4):
        out[0, b] = (res.results[2 * b]["out"]
                     + res.results[2 * b + 1]["out"]).reshape(N_T, N_IN)
    return out


# revision 22
# speedup vs baseline: 1.0036x; 1.0036x over previous
"""TRN2 kernel v5a: chunked causal linear attention, fp8-DR state/cross.

Beyond v2: the fp8 state path measured 1.1e-2 (< 2e-2 gate), so
 - state accumulates TWO chunks per fp8 DoubleRow matmul (kt = sub-chunk,
   K=256); V is still computed by fp16 matmuls but the psum is cast
   straight to fp8 V8 tiles (no fp16 V in SBUF at all) -- computing V
   from fp8 INPUTS instead measured 2.7e-2: quantize outputs, not inputs.
 - odd chunks replace the missing state tap with an unmasked cross
   Omega/intra block pair (exact math), so taps halve and chunk 1 loses
   its inter entirely.
PE: 340 matmuls (240 fp16 + 100 DoubleRow), ~133k columns
(v2: 356 / ~143k).  A PE-stream-only hardware probe ties the full kernel
within 1%: every copy/mask/tap/DMA is hidden and the remaining time is
columns x 0.42ns + ~58ns per matmul of dispatch + weight-load tax
(bass emits InstLdweights 1:1 with matmuls, no dedup).
"""

import numpy as np

N_T = 2048
N_IN = 256
C = 128
NCH = N_T // C      # 16 chunks, 8 pairs
T_TILE = 512
TT = N_T // T_TILE
HL = 4
N_CORES = 8

_cache = {}


def _tri_mask():
    idx = np.arange(128)
    return (idx[None, :] >= idx[:, None]).astype(np.float32)


def _build_nc(repeat=1, bf16="fp16"):
    import concourse.tile as tile
    import concourse.mybir as mybir
    from concourse import bacc

    F32 = mybir.dt.float32
    F8 = mybir.dt.float8e4
    FMM = mybir.dt.float16
    DR = mybir.MatmulPerfMode.DoubleRow

    nc = bacc.Bacc("TRN2", target_bir_lowering=False, debug=False,
                   num_devices=N_CORES)
    rT_d = nc.dram_tensor("rT", (2, 128, N_T), FMM, kind="ExternalInput").ap()
    rT8_d = nc.dram_tensor("rT8", (128, 2, N_T), F8,
                           kind="ExternalInput").ap()
    rn8_d = nc.dram_tensor("rn8", (128, NCH // 2, 2, N_IN), F8,
                           kind="ExternalInput").ap()
    Q4_d = nc.dram_tensor("Q4", (HL, 2, 128, N_IN), FMM,
                          kind="ExternalInput").ap()
    ET4_d = nc.dram_tensor("ET4", (HL, 2, 128, N_IN), FMM,
                           kind="ExternalInput").ap()
    mask_d = nc.dram_tensor("mask", (128, 128), FMM,
                            kind="ExternalInput").ap()
    out_d = nc.dram_tensor("out", (NCH, 128, N_IN), F32,
                           kind="ExternalOutput").ap()

    eng_load = {"v": 0.0, "s": 0.0}

    def copy_psum(out_ap, in_ap, n):
        dve = n / 0.96 + 150.0
        act = (n + 352.0) / 1.2
        if eng_load["v"] + dve <= eng_load["s"] + act:
            eng_load["v"] += dve
            nc.vector.tensor_copy(out_ap, in_ap)
        else:
            eng_load["s"] += act
            nc.scalar.copy(out_ap, in_ap)

    with tile.TileContext(nc) as tc:
        with (
            tc.tile_pool(name="const", bufs=1) as const,
            tc.tile_pool(name="om_pool", bufs=8) as om_pool,
            tc.tile_pool(name="opool", bufs=4) as opool,
            tc.tile_pool(name="psS", bufs=1, space="PSUM") as psS,
            tc.tile_pool(name="pswork", bufs=3, space="PSUM") as pswork,
            tc.tile_pool(name="psout", bufs=1, space="PSUM") as psout,
        ):
            warm_f32 = const.tile([128, 128], F32)
            nc.vector.memset(warm_f32, 0.0)
            warm_sb = const.tile([128, 128], FMM)
            nc.vector.tensor_copy(warm_sb, warm_f32)
            warm_ps = pswork.tile([128, T_TILE], F32, tag="w", name="warm_ps")
            for _w in range(24):
                nc.tensor.matmul(warm_ps[:, :128], warm_sb, warm_sb,
                                 start=True, stop=True, skip_group_check=True)

            mask_sb = const.tile([128, 2, 128], FMM)
            Q_h = [const.tile([128, 2, N_IN], FMM, name=f"Qh{h}")
                   for h in range(HL)]
            rT_sb = [const.tile([128, N_T], FMM, name=f"rT{jc}")
                     for jc in range(2)]
            rT8_sb = const.tile([128, 2, N_T], F8, name="rT8_sb")
            rn8_sb = const.tile([128, NCH // 2, 2, N_IN], F8, name="rn8_sb")
            ET_p = [const.tile([128, 2, 2, N_IN], FMM, name=f"ETp{p}")
                    for p in range(2)]
            for h in range(HL):
                for ic in range(2):
                    nc.sync.dma_start(out=Q_h[h][:, ic, :], in_=Q4_d[h, ic])
            for tq in range(TT):
                for ic in range(2):
                    nc.sync.dma_start(
                        out=rT_sb[ic][:, T_TILE * tq:T_TILE * (tq + 1)],
                        in_=rT_d[ic, :, T_TILE * tq:T_TILE * (tq + 1)])
            for jc in range(2):
                nc.sync.dma_start(out=rT8_sb[:, jc, :], in_=rT8_d[:, jc, :])
            for p in range(2):
                for jc in range(2):
                    for h2 in range(2):
                        nc.sync.dma_start(out=ET_p[p][:, jc, h2, :],
                                          in_=ET4_d[2 * p + h2, jc])
            for h2 in range(2):
                nc.sync.dma_start(out=mask_sb[:, h2, :], in_=mask_d)
            for m in range(NCH // 2):
                nc.sync.dma_start(out=rn8_sb[:, m, :, :], in_=rn8_d[:, m])

            # P^T fp16 [j, h2, n, t] (inter) + fp8 shadow [j, jc, h, n, t]
            PTp = [[const.tile([128, 2, NCH, C], FMM, name=f"PTp{p}_{jc}")
                    for jc in range(2)] for p in range(2)]
            PT8 = const.tile([128, 2, HL, NCH, C], F8, name="PT8")
            # V8 per (pair-of-chunks m, head-pair p): [u, kt=n%2, h2, i]
            V8_t = [[const.tile([128, 2, 2, N_IN], F8, name=f"V8{m}_{p}")
                     for p in range(2)] for m in range(NCH // 2)]
            S_ps = [[psS.tile([128, 2, N_IN], F32, name=f"Sps{p}_{jb}")
                     for jb in range(2)] for p in range(2)]
            S_sb = [[[const.tile([128, 2, N_IN], FMM, name=f"Ssb{p}_{jb}_{b}")
                      for b in range(2)] for jb in range(2)] for p in range(2)]
            # fp8 state taps [j, jb, h2, i] for the early fp8-DR inter chunks
            # (safe: |S| after <=8 chunks is far below e4m3 max)
            S8_sb = [[const.tile([128, 2, 2, N_IN], F8, name=f"S8sb{p}_{b}")
                      for b in range(2)] for p in range(2)]
            FP8_INTER_END = 9  # chunks 2..8 use fp8-DR inter (err 1.54e-2)

            def body():
                # ---- Phase A: P^T = Q^T rT (fp16, 2-pass over ic)
                for tq in range(TT):
                    ts = slice(T_TILE * tq, T_TILE * (tq + 1))
                    for p in range(2):
                        for h2 in range(2):
                            h = 2 * p + h2
                            for jc in range(2):
                                ps = pswork.tile([128, T_TILE], F32, tag="w",
                                                 name="ps_a")
                                for ic in range(2):
                                    nc.tensor.matmul(
                                        ps,
                                        Q_h[h][:, ic, 128 * jc:128 * (jc + 1)],
                                        rT_sb[ic][:, ts],
                                        start=(ic == 0), stop=(ic == 1))
                                copy_psum(
                                    PTp[p][jc][:, h2, 4 * tq:4 * tq + 4, :],
                                    ps, T_TILE)
                                copy_psum(
                                    PT8[:, jc, 2 * p + h2,
                                        4 * tq:4 * tq + 4, :],
                                    ps, T_TILE)
                # ---- Phase V: fp16 matmuls, psum cast straight to V8
                for n in range(NCH):
                    cs = slice(C * n, C * (n + 1))
                    pv = [pswork.tile([128, 2, N_IN], F32, tag="w",
                                      name=f"ps_v{p}") for p in range(2)]
                    for jc in range(2):
                        for p in range(2):
                            nc.tensor.matmul(pv[p], rT_sb[jc][:, cs],
                                             ET_p[p][:, jc, :, :],
                                             start=(jc == 0), stop=(jc == 1),
                                             skip_group_check=True)
                    for p in range(2):
                        copy_psum(V8_t[n // 2][p][:, n % 2, :, :], pv[p],
                                  2 * N_IN)
                # ---- Main loop ----
                for n in range(NCH):
                    cs = slice(C * n, C * (n + 1))
                    m = n // 2
                    odd = n % 2 == 1
                    # diag OmT (all 4 heads, one DR matmul)
                    w = pswork.tile([128, HL, C], F32, tag="w", name="ps_om")
                    nc.tensor.matmul(w, rT8_sb[:, :, cs],
                                     PT8[:, :, :, n, :],
                                     start=True, stop=True, perf_mode=DR,
                                     skip_group_check=True)
                    om = []
                    for p in range(2):
                        o = om_pool.tile([128, 2, C], F8, tag="om", name="om")
                        nc.vector.tensor_mul(o, w[:, 2 * p:2 * p + 2, :],
                                             mask_sb)
                        eng_load["v"] += 256 / 0.96 + 150.0
                        om.append(o)
                    if odd:  # cross OmT vs previous chunk's u (unmasked)
                        ps = slice(C * (n - 1), C * n)
                        wc = pswork.tile([128, HL, C], F32, tag="w",
                                         name="ps_omc")
                        nc.tensor.matmul(wc, rT8_sb[:, :, ps],
                                         PT8[:, :, :, n, :],
                                         start=True, stop=True, perf_mode=DR,
                                         skip_group_check=True)
                        omc = []
                        for p in range(2):
                            o = om_pool.tile([128, 2, C], F8, tag="om",
                                             name="omc")
                            copy_psum(o, wc[:, 2 * p:2 * p + 2, :], 2 * C)
                            omc.append(o)
                    po = psout.tile([128, N_IN], F32, tag="po", name="po")
                    fp8_inter = 2 <= n < FP8_INTER_END
                    n_mm = 2 + ((4 if fp8_inter else 8) if n >= 2 else 0) \
                        + (2 if odd else 0)
                    k = 0
                    if n >= 2:  # inter vs tap of pair (n-2)//2
                        bu = ((n - 2) // 2) % 2
                        for p in range(2):
                            for h2 in range(2):
                                if fp8_inter:  # one DR matmul, kt = jc
                                    nc.tensor.matmul(
                                        po, PT8[:, :, 2 * p + h2, n, :],
                                        S8_sb[p][bu][:, :, h2, :],
                                        start=(k == 0), stop=(k == n_mm - 1),
                                        perf_mode=DR, skip_group_check=True)
                                    k += 1
                                    continue
                                for jc in range(2):
                                    nc.tensor.matmul(
                                        po, PTp[p][jc][:, h2, n, :],
                                        S_sb[p][jc][bu][:, h2, :],
                                        start=(k == 0), stop=(k == n_mm - 1),
                                        skip_group_check=True)
                                    k += 1
                    for p in range(2):  # diag intra (DR head-pair sum)
                        nc.tensor.matmul(po, om[p], V8_t[m][p][:, n % 2, :, :],
                                         start=(k == 0), stop=(k == n_mm - 1),
                                         perf_mode=DR, skip_group_check=True)
                        k += 1
                    if odd:  # cross intra vs previous chunk's V
                        for p in range(2):
                            nc.tensor.matmul(po, omc[p],
                                             V8_t[m][p][:, 0, :, :],
                                             start=(k == 0),
                                             stop=(k == n_mm - 1),
                                             perf_mode=DR,
                                             skip_group_check=True)
                            k += 1
                    # state: add pair (n-1, n) at odd chunks 1..13 (DR, K=256)
                    if odd and n < NCH - 1:
                        for jb in range(2):
                            for p in range(2):
                                nc.tensor.matmul(
                                    S_ps[p][jb],
                                    rn8_sb[:, m, :, 128 * jb:128 * (jb + 1)],
                                    V8_t[m][p][:, :, :, :],
                                    start=(m == 0), stop=True,
                                    perf_mode=DR, skip_group_check=True)
                        for p in range(2):
                            for jb in range(2):
                                copy_psum(S_sb[p][jb][m % 2], S_ps[p][jb],
                                          2 * N_IN)
                        if m <= (FP8_INTER_END - 2) // 2:
                            for p in range(2):
                                for jb in range(2):
                                    copy_psum(S8_sb[p][m % 2][:, jb, :, :],
                                              S_ps[p][jb], 2 * N_IN)
                    ot = opool.tile([128, N_IN], F32, tag="ot", name="ot")
                    copy_psum(ot, po, N_IN)
                    nc.sync.dma_start(out=out_d[n], in_=ot)

            if repeat == 1:
                body()
            elif isinstance(repeat, tuple):
                loop_n, unroll_m = repeat
                with tc.For_i(0, loop_n, 1):
                    for _ in range(unroll_m):
                        body()
            elif repeat < 0:
                for _ in range(-repeat):
                    body()
            else:
                with tc.For_i(0, repeat, 1):
                    body()
    nc.compile()
    return nc


DTYPE = "fp16"


def _prep_in_maps(r_prime, E, Q, bf16="fp16"):
    import ml_dtypes
    f8 = ml_dtypes.float8_e4m3
    f16 = np.float16
    mask = _tri_mask()
    in_maps = []
    for c in range(N_CORES):
        b, hg = divmod(c, 2)
        heads = slice(4 * hg, 4 * hg + 4)
        rT = np.ascontiguousarray(r_prime[0, b].T).reshape(2, 128, N_T)
        rT8 = np.ascontiguousarray(rT.transpose(1, 0, 2))
        rn8 = np.ascontiguousarray(
            r_prime[0, b].reshape(NCH // 2, 2, 128, N_IN)
            .transpose(2, 0, 1, 3))
        Q4 = np.ascontiguousarray(Q[0, heads]).reshape(HL, 2, 128, N_IN)
        ET4 = np.ascontiguousarray(
            E[0, heads].transpose(0, 2, 1)).reshape(HL, 2, 128, N_IN)
        in_maps.append({"rT": rT.astype(f16),
                        "rT8": rT8.astype(f8),
                        "rn8": rn8.astype(f8),
                        "Q4": Q4.astype(f16),
                        "ET4": ET4.astype(f16),
                        "mask": mask.astype(f16)})
    return in_maps


def kernel(r_prime, E, Q):
    from concourse import bass_utils

    if "nc" not in _cache:
        _cache["nc"] = _build_nc(bf16=DTYPE)
    nc = _cache["nc"]
    in_maps = _prep_in_maps(r_prime, E, Q, bf16=DTYPE)
    res = bass_utils.run_bass_kernel_spmd(nc, in_maps,
                                          core_ids=list(range(N_CORES)))
    out = np.zeros((1, 4, N_T, N_IN), dtype=np.float32)
    for b in range(4):
        out[0, b] = (res.results[2 * b]["out"]
                     + res.results[2 * b + 1]["out"]).reshape(N_T, N_IN)
    return out
